# revision 1
# baseline (speedup 1.0000x reference)
"""Trainium2 Bass kernel for nn_CellGate (hetero GNN message passing + LSTM-style gate).

Strategy (8-core SPMD, dst-sharded):
- Each core owns a contiguous 12,500-node shard of both node types (A and B).
- Segment-mean aggregation per edge type via `dma_gather` passes: for each
  (edge-type, src-range, slot k) a gather pulls the k-th in-edge's source row
  for every destination in the range-local degree-sorted slot order, and DVE
  accumulates into an SBUF accumulator. int16 gather indices force 4 source
  ranges (2 shards each); per-range slot permutations give exact prefix
  widths (~3% padding). Range partials are merged into a DRAM master
  accumulator with `dma_scatter_add` (unique indices per chunk).
- Layer-0 gathers read the raw x table (host-relaid input); Wx is folded into
  the post-aggregation linears (mean is linear).
- Dense 64x64 linears: per-128-node-chunk PE transposes + matmuls in PSUM.
- One AllGather per node type rebuilds the full feature table between layers.
- Final tanh fused into PSUM evacuation; gates are elementwise on DVE.
"""

import numpy as np

import concourse.bass as bass
import concourse.bacc as bacc
import concourse.mybir as mybir
import concourse.tile as tile
from concourse.masks import make_identity

P = 128
D = 64

# edge types: (name, src_type, dst_type)
ETS = [("AB", 0, 1), ("BA", 1, 0), ("AA", 0, 0)]
L = 2

# width register value set (multiples of 128); must include scatter chunk sizes
WSET_G = [1, 2, 3, 4, 5, 6, 8, 10, 12, 14, 16, 20, 24, 25, 28, 32, 40, 48, 49,
          56, 64, 80, 98]


def full_cfg():
    return dict(n_cores=8, shard=12500, G=98, n_ranges=4, spr=2)


def cfg_derived(cfg):
    c = dict(cfg)
    c["pad"] = P * c["G"]
    c["rwin"] = c["spr"] * c["pad"]
    c["trows"] = c["n_cores"] * c["pad"]
    c["nnodes"] = c["n_cores"] * c["shard"]
    c["wset"] = [g * P for g in WSET_G if g <= c["G"]]
    if c["G"] not in [w // P for w in c["wset"]]:
        c["wset"].append(c["G"] * P)
    # scatter chunk widths
    sch = []
    g0 = 0
    while g0 < c["G"]:
        g1 = min(g0 + 25, c["G"])
        sch.append((g0, g1))
        g0 = g1
    c["scatter_chunks"] = sch
    for (g0, g1) in sch:
        w = (g1 - g0) * P
        if w not in c["wset"]:
            c["wset"].append(w)
    c["wset"] = sorted(set(c["wset"]))
    return c


def roundw(w, wset):
    for v in wset:
        if v >= w:
            return v
    return wset[-1]


# ---------------------------------------------------------------- host prep

def node_to_row(n, shard, pad):
    c = n // shard
    return pad * c + (n - shard * c)


def host_prep(cfg, edges):
    """edges: dict name -> [2, E] int32 (src, dst global).

    Returns: schedule (static, shared): list of passes
      (et_i, r, k, w)  with w rounded to wset
    and per-core arrays:
      gidx[core]: int16 [128, CBTOT]  (packed gather indices, 8-replicated)
      sidx[core]: int16 [3, n_ranges, 128, pad//16]
      deg[core]:  int32 [3, pad]
    """
    cfg = cfg_derived(cfg)
    NCO, SH, PAD, G = cfg["n_cores"], cfg["shard"], cfg["pad"], cfg["G"]
    NR, RWIN, WSET = cfg["n_ranges"], cfg["rwin"], cfg["wset"]

    # per (core, et, r): slot order theta_r (positions sorted by deg_r desc)
    # and per-slot src-row lists.
    percore = [dict(gidx_parts=[], sidx=np.zeros((3, NR, 128, PAD // 16), np.int16),
                    deg=np.zeros((3, PAD), np.int32)) for _ in range(NCO)]

    # first compute per-(core,et,r) CSR by slot, find per-pass counts
    all_counts = {}   # (et_i, r, k) -> max over cores of count
    maxk = {}         # (et_i, r) -> K
    core_data = {}    # (core, et_i, r) -> (theta, srcrows_by_k list of arrays)
    for et_i, (etn, sT, dT) in enumerate(ETS):
        src, dst = edges[etn][0].astype(np.int64), edges[etn][1].astype(np.int64)
        srow = PAD * (src // SH) + (src - SH * (src // SH))  # global table row
        for c in range(NCO):
            m = (dst // SH) == c
            s_r, d_l = srow[m], dst[m] - c * SH
            percore[c]["deg"][et_i] = np.bincount(d_l, minlength=PAD)[:PAD]
            for r in range(NR):
                rm = (s_r // RWIN) == r
                sl, dl = s_r[rm] - r * RWIN, d_l[rm]
                deg_r = np.bincount(dl, minlength=PAD)[:PAD]
                theta = np.argsort(-deg_r, kind="stable")
                slot_of = np.empty(PAD, np.int64)
                slot_of[theta] = np.arange(PAD)
                K = int(deg_r.max()) if deg_r.size else 0
                maxk[(et_i, r)] = max(maxk.get((et_i, r), 0), K)
                # order edges by (dst, arbitrary) -> kth edge per dst
                order = np.argsort(dl, kind="stable")
                dls, sls = dl[order], sl[order]
                # rank within dst group
                if dls.size:
                    starts = np.r_[0, np.nonzero(np.diff(dls))[0] + 1]
                    rank = np.arange(dls.size) - np.repeat(starts, np.diff(np.r_[starts, dls.size]))
                else:
                    rank = np.zeros(0, np.int64)
                by_k = []
                for k in range(K):
                    km = rank == k
                    dk, sk = dls[km], sls[km]
                    cnt = dk.size
                    all_counts[(et_i, r, k)] = max(all_counts.get((et_i, r, k), 0), cnt)
                    by_k.append((slot_of[dk], sk))
                core_data[(c, et_i, r)] = (theta, by_k)
                # scatter idx: slot -> position (theta), 16-wrapped, 8-replicated
                w = theta.astype(np.int16).reshape(PAD // 16, 16).T
                percore[c]["sidx"][et_i, r] = np.tile(w, (8, 1))

    # static schedule with rounded widths, split into pieces of <= WCAP groups
    WCAP_G = 49
    schedule = []
    for et_i in range(3):
        for r in range(NR):
            for k in range(maxk.get((et_i, r), 0)):
                w = roundw(max(all_counts.get((et_i, r, k), 1), 1), WSET)
                go = 0
                while go * P < w:
                    gw = min(WCAP_G, w // P - go)
                    wp = roundw(gw * P, WSET)
                    assert wp // P <= WCAP_G, (w, go, gw, wp)
                    schedule.append((et_i, r, k, go, wp))
                    go += wp // P
    # Group per (et, r) so each accumulator flushes exactly once per range;
    # round-robin across the 3 edge types for cross-accumulator pipelining.
    per_et = [[] for _ in range(3)]
    for t in sorted(schedule, key=lambda t: (t[0], t[1], t[2], t[3])):
        per_et[t[0]].append(t)
    schedule = []
    i = [0, 0, 0]
    while any(i[e] < len(per_et[e]) for e in range(3)):
        for e in range(3):
            if i[e] < len(per_et[e]):
                schedule.append(per_et[e][i[e]])
                i[e] += 1

    # build packed gather idx per core
    ZLOC = SH  # zero-row local index within each range window (first shard's pad)
    for c in range(NCO):
        # build full-width pass arrays once per (et, r, k), then slice pieces
        full_arr = {}
        for (et_i, r, k, go, wp) in schedule:
            key = (c, et_i, r, k)
            if key not in full_arr:
                theta, by_k = core_data[(c, et_i, r)]
                arr = np.full(PAD, ZLOC, np.int16)
                if k < len(by_k):
                    slots, srcs = by_k[k]
                    arr[slots] = srcs.astype(np.int16)
                full_arr[key] = arr
        parts = []
        for (et_i, r, k, go, wp) in schedule:
            arr = np.full(wp, ZLOC, np.int16)
            seg = full_arr[(c, et_i, r, k)][go * P:go * P + wp]
            arr[:seg.size] = seg
            wrapped = arr.reshape(wp // 16, 16).T  # [16, wp/16]
            parts.append(np.tile(wrapped, (8, 1)))  # [128, wp/16]
        percore[c]["gidx"] = np.concatenate(parts, axis=1)
    cbtot = percore[0]["gidx"].shape[1]
    return cfg, schedule, percore, cbtot


# ---------------------------------------------------------------- builder

def build(cfg, schedule, cbtot, skip_gather=False, skip_dense=False, skip_cc=False, skip_scatter=False, scatter_sp=True):
    cfg = cfg_derived(cfg)
    NCO, SH, PAD, G = cfg["n_cores"], cfg["shard"], cfg["pad"], cfg["G"]
    NR, RWIN, TROWS, WSET = cfg["n_ranges"], cfg["rwin"], cfg["trows"], cfg["wset"]
    f32 = mybir.dt.float32
    i32 = mybir.dt.int32
    i16 = mybir.dt.int16

    nc = bacc.Bacc(None, target_bir_lowering=False, debug=False,
                   num_swdge_queues=4, num_devices=NCO)

    # ---------------- inputs
    tabx = [nc.declare_dram_parameter(f"tabx{t}", [TROWS, D], f32, isOutput=False)
            for t in "AB"]
    xsh = [nc.declare_dram_parameter(f"x{t}", [PAD, D], f32, isOutput=False)
           for t in "AB"]
    gates_in = {}
    for t in "AB":
        for nmm in "cif":
            gates_in[nmm + t] = nc.declare_dram_parameter(
                f"{nmm}{t}", [PAD, D], f32, isOutput=False)
    wx = [nc.declare_dram_parameter(f"wx{t}", [D, D], f32, isOutput=False)
          for t in "AB"]
    wlt = nc.declare_dram_parameter("wlt", [L, 3, D, D], f32, isOutput=False)
    wrt = nc.declare_dram_parameter("wrt", [L, 3, D, D], f32, isOutput=False)
    blc = nc.declare_dram_parameter("blc", [D, L, 3], f32, isOutput=False)
    biasc = nc.declare_dram_parameter("biasc", [D, 2], f32, isOutput=False)
    deg_in = nc.declare_dram_parameter("deg", [3, PAD], i32, isOutput=False)
    gidx = nc.declare_dram_parameter("gidx", [128, cbtot], i16, isOutput=False)
    sidx = nc.declare_dram_parameter("sidx", [3, NR, 128, PAD // 16], i16,
                                     isOutput=False)
    outs = [nc.declare_dram_parameter(f"out{t}", [PAD, D], f32, isOutput=True)
            for t in "AB"]

    # ---------------- DRAM internals
    masters = [[nc.dram_tensor(f"m{ETS[e][0]}_{l}", [PAD, D], f32)
                for e in range(3)] for l in range(L)]
    stg = [nc.dram_tensor(f"stg{t}", [PAD, D], f32) for t in "AB"]
    tab_space = "Shared" if NCO > 4 else "Local"
    tab1 = [nc.dram_tensor(f"tab1{t}", [TROWS, D], f32, addr_space=tab_space)
            for t in "AB"]

    # ---------------- width registers (before TileContext)
    wregs = {}
    for w in WSET:
        r = nc.alloc_register(mybir.EngineType.Pool, f"w{w}")
        nc.gpsimd.reg_mov(r, w)
        wregs[w] = r

    rearr = "(p g) d -> p g d"

    with tile.TileContext(nc) as tc:
        with tc.tile_pool(name="const", bufs=1) as cpool, \
             tc.tile_pool(name="tsh", bufs=1) as tpool, \
             tc.tile_pool(name="accp", bufs=1) as apool, \
             tc.tile_pool(name="idxp", bufs=6) as ipool, \
             tc.tile_pool(name="sidxp", bufs=2) as spool, \
             tc.tile_pool(name="msgp", bufs=3) as mpool, \
             tc.tile_pool(name="densep", bufs=3) as dpool, \
             tc.tile_pool(name="psA", bufs=2, space="PSUM") as psA, \
             tc.tile_pool(name="psB", bufs=2, space="PSUM") as psB, \
             tc.tile_pool(name="psT", bufs=2, space="PSUM") as psT, \
             tc.tile_pool(name="psK", bufs=2, space="PSUM") as psK:

            # ---- constants
            ident = cpool.tile([P, P], f32)
            make_identity(nc, ident[:])
            wlt_t = cpool.tile([D, L * 3, D], f32)
            wrt_t = cpool.tile([D, L * 3, D], f32)
            nc.sync.dma_start(out=wlt_t[:], in_=wlt[:].rearrange("l e a b -> a (l e) b"))
            nc.sync.dma_start(out=wrt_t[:], in_=wrt[:].rearrange("l e a b -> a (l e) b"))
            wx_t = cpool.tile([D, 2, D], f32)  # original Wx (lhsT for fold)
            for t in range(2):
                nc.sync.dma_start(out=wx_t[:, t, :], in_=wx[t][:])
            blc_t = cpool.tile([D, L, 3], f32)
            nc.sync.dma_start(out=blc_t[:], in_=blc[:])
            biasc_t = cpool.tile([D, 2], f32)
            nc.sync.dma_start(out=biasc_t[:], in_=biasc[:])

            blA = cpool.tile([D, L], f32)
            for l in range(L):
                nc.vector.tensor_add(out=blA[:, l:l + 1], in0=blc_t[:, l, 1:2],
                                     in1=blc_t[:, l, 2:3])
            fbA = cpool.tile([D, 1], f32)
            fbB = cpool.tile([D, 1], f32)
            nc.vector.tensor_add(out=fbA[:], in0=blA[:, L - 1:L], in1=biasc_t[:, 0:1])
            nc.vector.tensor_add(out=fbB[:], in0=blc_t[:, L - 1, 0:1], in1=biasc_t[:, 1:2])

            # layer-0 folded weights: W'^T = Wx^T @ W^T (lhsT=Wx, rhs=W^T)
            wl0f = cpool.tile([D, 3, D], f32)
            wr0f = cpool.tile([D, 3, D], f32)
            for e, (_, sT, dT) in enumerate(ETS):
                pw = psT.tile([D, D], f32, tag="tr", name="pw")
                nc.tensor.matmul(out=pw[:], lhsT=wx_t[:, sT, :], rhs=wlt_t[:, e, :],
                                 start=True, stop=True)
                nc.vector.tensor_copy(out=wl0f[:, e, :], in_=pw[:])
                pw2 = psT.tile([D, D], f32, tag="tr", name="pw2")
                nc.tensor.matmul(out=pw2[:], lhsT=wx_t[:, dT, :], rhs=wrt_t[:, e, :],
                                 start=True, stop=True)
                nc.vector.tensor_copy(out=wr0f[:, e, :], in_=pw2[:])

            # deg -> recip [128, 3, G]
            deg_t = cpool.tile([P, 3, G], i32)
            nc.sync.dma_start(out=deg_t[:], in_=deg_in[:].rearrange("e (p g) -> p e g", p=P))
            recip = cpool.tile([P, 3, G], f32)
            nc.vector.tensor_copy(out=recip[:], in_=deg_t[:])
            nc.vector.tensor_scalar_max(recip[:], recip[:], 1.0)
            nc.vector.reciprocal(out=recip[:], in_=recip[:])

            zero_small = cpool.tile([P, D], f32)
            nc.vector.memset(zero_small[:], 0.0)

            # t shards (layer outputs), node-major
            t_t = [tpool.tile([P, G, D], f32, tag=f"t{t}", name=f"t_t{t}") for t in range(2)]
            if skip_dense:
                for t in range(2):
                    nc.vector.memset(t_t[t][:], 0.5)

            # ============ per layer ============
            for l in range(L):
                tabs = [tabx[0], tabx[1]] if l == 0 else [tab1[0], tab1[1]]

                accs = [apool.tile([P, G, D], f32, tag=f"acc{e}", name=f"acc_{l}_{e}")
                        for e in range(3)]
                cur_r = [0, 0, 0]
                for e in range(3):
                    nc.vector.memset(accs[e][:], 0.0)
                    if l == 0:
                        # masters start zero: write the freshly-zeroed acc out
                        for ll in range(L):
                            nc.sync.dma_start(
                                out=masters[ll][e][:].rearrange(rearr, p=P),
                                in_=accs[e][:])

                def flush(e, r, accs=accs, l=l):
                    sx = spool.tile([P, PAD // 16], i16, tag="sx", name=f"sx_{l}_{e}_{r}")
                    nc.sync.dma_start(out=sx[:], in_=sidx[e, r])
                    for (g0, g1) in cfg["scatter_chunks"]:
                        if skip_scatter:
                            continue
                        w = (g1 - g0) * P
                        nc.gpsimd.dma_scatter_add(
                            masters[l][e][:], accs[e][:, g0:g1, :],
                            sx[:, g0 * 8:g1 * 8], w, wregs[w], D,
                            single_packet=scatter_sp, queue_num=0)

                col = 0
                qn = 0
                for (e, r, k, go, wp) in schedule:
                    if r != cur_r[e]:
                        flush(e, cur_r[e])
                        cur_r[e] = r
                        accs[e] = apool.tile([P, G, D], f32, tag=f"acc{e}",
                                             name=f"acc_{l}_{e}_r{r}")
                        nc.vector.memset(accs[e][:], 0.0)
                    cb = wp // 16
                    gw = wp // P
                    idx_t = ipool.tile([P, cb], i16, tag="gi", name=f"gi_{l}_{qn}")
                    nc.sync.dma_start(out=idx_t[:], in_=gidx[:, col:col + cb])
                    msg = mpool.tile([P, gw, D], f32, tag="msg", name=f"msg_{l}_{qn}")
                    sT = ETS[e][1]
                    if not skip_gather:
                        nc.gpsimd.dma_gather(
                            out_ap=msg[:],
                            in_ap=tabs[sT][r * RWIN:(r + 1) * RWIN, :],
                            idxs_ap=idx_t[:],
                            num_idxs=wp, num_idxs_reg=wregs[wp], elem_size=D,
                            single_packet=False, queue_num=qn % 4)
                        nc.vector.tensor_add(out=accs[e][:, go:go + gw, :],
                                             in0=accs[e][:, go:go + gw, :], in1=msg[:])
                    qn += 1
                    col += cb
                for e in range(3):
                    flush(e, cur_r[e])

                # ---- dense stage: load masters back into the acc-tagged slots
                msrc = [apool.tile([P, G, D], f32, tag=f"acc{e}", name=f"msrc_{l}_{e}")
                        for e in range(3)]
                for e in range(3):
                    nc.sync.dma_start(out=msrc[e][:],
                                      in_=masters[l][e][:].rearrange(rearr, p=P))
                wl_use = wl0f if l == 0 else wlt_t
                wr_use = wr0f if l == 0 else wrt_t
                woff = 0 if l == 0 else l * 3
                last = (l == L - 1)

                for g in (range(G) if not skip_dense else []):
                    for e in range(3):
                        nc.vector.tensor_scalar_mul(
                            msrc[e][:, g, :], msrc[e][:, g, :], recip[:, e, g:g + 1])
                    # t-source chunks: layer 0 streams x from DRAM, layer 1 uses t_t
                    if l == 0:
                        tch = []
                        for t in range(2):
                            xc = dpool.tile([P, D], f32, tag=f"xc{t}", name=f"xc_{g}_{t}")
                            nc.sync.dma_start(
                                out=xc[:],
                                in_=xsh[t][:].rearrange(rearr, p=P)[:, g, :])
                            tch.append(xc[:])
                    else:
                        tch = [t_t[0][:, g, :], t_t[1][:, g, :]]
                    trs = []
                    for src_ap in (msrc[0][:, g, :], msrc[1][:, g, :], msrc[2][:, g, :],
                                   tch[0], tch[1]):
                        pt = psT.tile([D, P], f32, tag="tr", name=f"pt_{g}")
                        nc.tensor.transpose(out=pt[:], in_=src_ap, identity=ident[:])
                        st = dpool.tile([D, P], f32, tag="trs", name=f"st_{g}")
                        nc.vector.tensor_copy(out=st[:], in_=pt[:])
                        trs.append(st)
                    sAB, sBA, sAA, sxA, sxB = trs
                    pA = psA.tile([D, P], f32, tag="pa", name=f"pA_{g}")
                    nc.tensor.matmul(out=pA[:], lhsT=wl_use[:, woff + 1, :], rhs=sBA[:], start=True, stop=False)
                    nc.tensor.matmul(out=pA[:], lhsT=wl_use[:, woff + 2, :], rhs=sAA[:], start=False, stop=False)
                    nc.tensor.matmul(out=pA[:], lhsT=wr_use[:, woff + 1, :], rhs=sxA[:], start=False, stop=False)
                    nc.tensor.matmul(out=pA[:], lhsT=wr_use[:, woff + 2, :], rhs=sxA[:], start=False, stop=True)
                    pB = psB.tile([D, P], f32, tag="pb", name=f"pB_{g}")
                    nc.tensor.matmul(out=pB[:], lhsT=wl_use[:, woff + 0, :], rhs=sAB[:], start=True, stop=False)
                    nc.tensor.matmul(out=pB[:], lhsT=wr_use[:, woff + 0, :], rhs=sxB[:], start=False, stop=True)
                    nA = dpool.tile([D, P], f32, tag="nA", name=f"nA_{g}")
                    nB = dpool.tile([D, P], f32, tag="nB", name=f"nB_{g}")
                    if last:
                        nc.scalar.activation(nA[:], pA[:], mybir.ActivationFunctionType.Tanh,
                                             bias=fbA[:, 0:1])
                        nc.scalar.activation(nB[:], pB[:], mybir.ActivationFunctionType.Tanh,
                                             bias=fbB[:, 0:1])
                    else:
                        nc.vector.tensor_scalar_add(nA[:], pA[:], blA[:, l:l + 1])
                        nc.vector.tensor_scalar_add(nB[:], pB[:], blc_t[:, l, 0:1])
                    for nsb, tt in ((nA, t_t[0]), (nB, t_t[1])):
                        pback = psK.tile([P, D], f32, tag="back", name=f"pk_{g}")
                        nc.tensor.transpose(out=pback[:], in_=nsb[:], identity=ident[:D, :D])
                        nc.vector.tensor_copy(out=tt[:, g, :], in_=pback[:])

                if not last:
                    for t in range(2):
                        nc.sync.dma_start(out=stg[t][:].rearrange(rearr, p=P), in_=t_t[t][:])
                        if PAD > SH:
                            nc.sync.dma_start(out=stg[t][SH:PAD, :],
                                              in_=zero_small[0:PAD - SH, :])
                        if skip_cc:
                            nc.sync.dma_start(out=tab1[t][0:PAD, :], in_=stg[t][:])
                        else:
                            nc.gpsimd.collective_compute(
                                "AllGather", mybir.AluOpType.bypass,
                                replica_groups=[list(range(NCO))],
                                ins=[stg[t][:]], outs=[tab1[t][:]])

            # ---- gates: out = f*c + i*tanh_t  (reuse acc-tagged slots)
            for t in range(2):
                ct = apool.tile([P, G, D], f32, tag="acc0", name=f"ct{t}")
                it = apool.tile([P, G, D], f32, tag="acc1", name=f"it{t}")
                ft = apool.tile([P, G, D], f32, tag="acc2", name=f"ft{t}")
                tn = "AB"[t]
                nc.sync.dma_start(out=ct[:], in_=gates_in["c" + tn][:].rearrange(rearr, p=P))
                nc.sync.dma_start(out=it[:], in_=gates_in["i" + tn][:].rearrange(rearr, p=P))
                nc.sync.dma_start(out=ft[:], in_=gates_in["f" + tn][:].rearrange(rearr, p=P))
                nc.vector.tensor_mul(out=ft[:], in0=ft[:], in1=ct[:])
                nc.vector.tensor_mul(out=it[:], in0=it[:], in1=t_t[t][:])
                nc.vector.tensor_add(out=ft[:], in0=ft[:], in1=it[:])
                nc.sync.dma_start(out=outs[t][:].rearrange(rearr, p=P), in_=ft[:])

    # Align SWDGE queue_num with Tile's DMASW semaphore lane assignment:
    # each DMASW sem must only ever be updated from one SWDGE queue, and
    # Tile assigns lanes round-robin over the scheduled order. queue = lane%4.
    import re as _re
    for _ins in list(nc.inst_map.values()):
        if isinstance(_ins, (mybir.InstDMAGatherAnt, mybir.InstDMAScatterAddAnt)):
            _si = _ins.sync_info
            for _u in (_si.on_update or []):
                _m = _re.match(r"DMASW(\d+)", getattr(_u, "ant_name", "") or "")
                if _m:
                    _ins.queue_num = int(_m.group(1)) % 4
                    break

    nc.compile()
    return nc


# ---------------------------------------------------------------- host wrapper

def make_in_maps(cfg, inputs, percore):
    cfg = cfg_derived(cfg)
    NCO, SH, PAD, TROWS = cfg["n_cores"], cfg["shard"], cfg["pad"], cfg["trows"]

    def pad_rows(a):
        out = np.zeros((PAD, D), np.float32)
        out[:SH] = a
        return out

    # full x in table layout
    tabx = {}
    for t, xn in (("A", "x_A"), ("B", "x_B")):
        tb = np.zeros((TROWS, D), np.float32)
        x = np.asarray(inputs[xn], np.float32)
        for c in range(NCO):
            tb[PAD * c:PAD * c + SH] = x[SH * c:SH * (c + 1)]
        tabx[t] = tb

    wlt = np.ascontiguousarray(np.swapaxes(np.asarray(inputs["Wl"], np.float32), 2, 3))
    wrt = np.ascontiguousarray(np.swapaxes(np.asarray(inputs["Wr"], np.float32), 2, 3))
    blc = np.ascontiguousarray(np.transpose(np.asarray(inputs["bl"], np.float32), (2, 0, 1)))
    biasc = np.ascontiguousarray(
        np.stack([np.asarray(inputs["bias_A"], np.float32),
                  np.asarray(inputs["bias_B"], np.float32)], axis=1))

    in_maps = []
    for c in range(NCO):
        sl = slice(SH * c, SH * (c + 1))
        m = {
            "tabxA": tabx["A"], "tabxB": tabx["B"],
            "xA": pad_rows(np.asarray(inputs["x_A"])[sl]),
            "xB": pad_rows(np.asarray(inputs["x_B"])[sl]),
            "wxA": np.asarray(inputs["Wx_A"], np.float32),
            "wxB": np.asarray(inputs["Wx_B"], np.float32),
            "wlt": wlt, "wrt": wrt, "blc": blc, "biasc": biasc,
            "deg": percore[c]["deg"],
            "gidx": percore[c]["gidx"],
            "sidx": percore[c]["sidx"],
        }
        for t in "AB":
            for nmm in "cif":
                m[f"{nmm}{t}"] = pad_rows(np.asarray(inputs[f"{nmm}_{t}"])[sl])
        in_maps.append(m)
    return in_maps


_BUILT = {}


def kernel(**inputs):
    from concourse.bass_utils import run_bass_kernel_spmd

    cfg0 = full_cfg()
    edges = {"AB": np.asarray(inputs["edge_AB"]),
             "BA": np.asarray(inputs["edge_BA"]),
             "AA": np.asarray(inputs["edge_AA"])}
    cfg, schedule, percore, cbtot = host_prep(cfg0, edges)

    key = (cbtot, tuple(schedule))
    if key not in _BUILT:
        _BUILT.clear()
        _BUILT[key] = build(cfg0, schedule, cbtot)
    nc = _BUILT[key]

    in_maps = make_in_maps(cfg0, inputs, percore)
    r = run_bass_kernel_spmd(nc, in_maps, core_ids=list(range(cfg["n_cores"])))

    SH = cfg["shard"]
    out_A = np.concatenate([r.results[c]["outA"][:SH] for c in range(cfg["n_cores"])], axis=0)
    out_B = np.concatenate([r.results[c]["outB"][:SH] for c in range(cfg["n_cores"])], axis=0)
    return (out_A, out_B)



# revision 12
# speedup vs baseline: 1.1783x; 1.1783x over previous
"""Trainium2 Bass kernel for nn_CellGate (hetero GNN message passing + LSTM-style gate).

Strategy (8-core SPMD, dst-sharded), v2:
- Each core owns a contiguous 12,500-node shard of both node types (A and B).
- Segment-mean aggregation per edge type via `dma_gather` slot passes (as v1),
  but with small pass widths (<=24 groups), deep idx/msg pools and round-robin
  SWDGE queues so all four Q7 descriptor-generation pairs run concurrently.
- Per-(et,range) flush: one DVE cast f32->bf16 of the slot accumulator, then 4
  `dma_scatter_add`s (bf16, 256B rows) into a host-zeroed DRAM master.
- Dense stage is transpose-free: masters are read back with HWDGE DMA-transpose
  into feature-major bf16 [64, nodes] tiles, recip (mean) applied there once,
  and per-128-node-chunk matmuls consume the same feature-major tile as lhsT
  (node-major output) and as rhs (feature-major output for the next layer's
  W_r term). Biases ride in via partition-replicated adds / activation bias.
- Weights (incl. the Wx input-projection folds for layer 0) are folded on host
  and shipped bf16.
- One AllGather per node type rebuilds the full f32 gather table between
  layers; gates are elementwise on DVE at the end.
"""

import numpy as np

import concourse.bass as bass
import concourse.bacc as bacc
import concourse.mybir as mybir
import concourse.tile as tile

P = 128
D = 64

# edge types: (name, src_type, dst_type)
ETS = [("AB", 0, 1), ("BA", 1, 0), ("AA", 0, 0)]
L = 2

WCAP_G = 24              # max gather pass width in groups of 128
WSET_G = list(range(1, 26))   # width register values (multiples of 128)


def full_cfg():
    return dict(n_cores=8, shard=12500, G=98, n_ranges=4, spr=2)


def cfg_derived(cfg):
    c = dict(cfg)
    c["pad"] = P * c["G"]
    c["rwin"] = c["spr"] * c["pad"]
    c["trows"] = c["n_cores"] * c["pad"]
    c["nnodes"] = c["n_cores"] * c["shard"]
    c["wset"] = [g * P for g in WSET_G]
    # scatter chunk group ranges (4 chunks per flush)
    sch = []
    g0 = 0
    while g0 < c["G"]:
        g1 = min(g0 + 25, c["G"])
        sch.append((g0, g1))
        g0 = g1
    c["scatter_chunks"] = sch
    return c


def roundw(w, wset):
    for v in wset:
        if v >= w:
            return v
    return wset[-1]


# ---------------------------------------------------------------- host prep

def host_prep(cfg, edges):
    """edges: dict name -> [2, E] int32 (src, dst global).

    Returns: schedule (static, shared): list of passes (et_i, r, k, go, wp)
    and per-core arrays:
      gidx[core]: int16 [128, CBTOT]  (packed gather indices, 8-replicated)
      sidx[core]: int16 [3, n_ranges, 128, pad//16]
      deg[core]:  int32 [3, pad]   (total in-degree per et)
    """
    cfg = cfg_derived(cfg)
    NCO, SH, PAD, G = cfg["n_cores"], cfg["shard"], cfg["pad"], cfg["G"]
    NR, RWIN, WSET = cfg["n_ranges"], cfg["rwin"], cfg["wset"]

    percore = [dict(sidx=np.zeros((3, NR, 128, PAD // 16), np.int16),
                    deg=np.zeros((3, PAD), np.int32)) for _ in range(NCO)]

    all_counts = {}   # (et_i, r, k) -> max over cores of count
    maxk = {}         # (et_i, r) -> K
    core_data = {}    # (core, et_i, r) -> (theta, by_k list)
    for et_i, (etn, sT, dT) in enumerate(ETS):
        src, dst = edges[etn][0].astype(np.int64), edges[etn][1].astype(np.int64)
        srow = PAD * (src // SH) + (src - SH * (src // SH))  # global table row
        for c in range(NCO):
            m = (dst // SH) == c
            s_r, d_l = srow[m], dst[m] - c * SH
            percore[c]["deg"][et_i] = np.bincount(d_l, minlength=PAD)[:PAD]
            for r in range(NR):
                rm = (s_r // RWIN) == r
                sl, dl = s_r[rm] - r * RWIN, d_l[rm]
                deg_r = np.bincount(dl, minlength=PAD)[:PAD]
                theta = np.argsort(-deg_r, kind="stable")
                slot_of = np.empty(PAD, np.int64)
                slot_of[theta] = np.arange(PAD)
                K = int(deg_r.max()) if deg_r.size else 0
                maxk[(et_i, r)] = max(maxk.get((et_i, r), 0), K)
                order = np.argsort(dl, kind="stable")
                dls, sls = dl[order], sl[order]
                if dls.size:
                    starts = np.r_[0, np.nonzero(np.diff(dls))[0] + 1]
                    rank = np.arange(dls.size) - np.repeat(
                        starts, np.diff(np.r_[starts, dls.size]))
                else:
                    rank = np.zeros(0, np.int64)
                by_k = []
                for k in range(K):
                    km = rank == k
                    dk, sk = dls[km], sls[km]
                    all_counts[(et_i, r, k)] = max(
                        all_counts.get((et_i, r, k), 0), dk.size)
                    by_k.append((slot_of[dk], sk))
                core_data[(c, et_i, r)] = (theta, by_k)
                w = theta.astype(np.int16).reshape(PAD // 16, 16).T
                percore[c]["sidx"][et_i, r] = np.tile(w, (8, 1))

    # static schedule, pieces of <= WCAP_G groups, round-robin across ets
    schedule = []
    for et_i in range(3):
        for r in range(NR):
            for k in range(maxk.get((et_i, r), 0)):
                w = ((max(all_counts.get((et_i, r, k), 1), 1) + P - 1) // P) * P
                go = 0
                while go * P < w:
                    gw = min(WCAP_G, w // P - go)
                    wp = roundw(gw * P, WSET)
                    schedule.append((et_i, r, k, go, wp))
                    go += wp // P
    per_et = [[] for _ in range(3)]
    for t in sorted(schedule, key=lambda t: (t[0], t[1], t[2], t[3])):
        per_et[t[0]].append(t)
    schedule = []
    i = [0, 0, 0]
    while any(i[e] < len(per_et[e]) for e in range(3)):
        for e in range(3):
            if i[e] < len(per_et[e]):
                schedule.append(per_et[e][i[e]])
                i[e] += 1

    # packed gather idx per core
    ZLOC = SH  # zero-row local index within each range window
    for c in range(NCO):
        full_arr = {}
        for (et_i, r, k, go, wp) in schedule:
            key = (c, et_i, r, k)
            if key not in full_arr:
                theta, by_k = core_data[(c, et_i, r)]
                arr = np.full(PAD, ZLOC, np.int16)
                if k < len(by_k):
                    slots, srcs = by_k[k]
                    arr[slots] = srcs.astype(np.int16)
                full_arr[key] = arr
        parts = []
        for (et_i, r, k, go, wp) in schedule:
            arr = np.full(wp, ZLOC, np.int16)
            seg = full_arr[(c, et_i, r, k)][go * P:go * P + wp]
            arr[:seg.size] = seg
            wrapped = arr.reshape(wp // 16, 16).T
            parts.append(np.tile(wrapped, (8, 1)))
        percore[c]["gidx"] = np.concatenate(parts, axis=1)
    cbtot = percore[0]["gidx"].shape[1]
    return cfg, schedule, percore, cbtot


# ---------------------------------------------------------------- builder

def build(cfg, schedule, cbtot, skip_gather=False, skip_dense=False,
          skip_cc=False, skip_scatter=False):
    cfg = cfg_derived(cfg)
    NCO, SH, PAD, G = cfg["n_cores"], cfg["shard"], cfg["pad"], cfg["G"]
    NR, RWIN, TROWS, WSET = cfg["n_ranges"], cfg["rwin"], cfg["trows"], cfg["wset"]
    f32 = mybir.dt.float32
    bf16 = mybir.dt.bfloat16
    i16 = mybir.dt.int16
    HG = 49                    # groups per dense half
    HR = HG * P                # rows per dense half
    BL = 4                     # dense groups per block
    GCHUNK = 20                # gates groups per chunk

    nc = bacc.Bacc(None, target_bir_lowering=False, debug=False,
                   num_swdge_queues=4, num_devices=NCO)

    # ---------------- inputs
    tabx = [nc.declare_dram_parameter(f"tabx{t}", [TROWS, D], f32, isOutput=False)
            for t in "AB"]
    xf = [nc.declare_dram_parameter(f"xf{t}", [D, PAD], bf16, isOutput=False)
          for t in "AB"]
    gates_in = {}
    for t in "AB":
        for nmm in "cif":
            gates_in[nmm + t] = nc.declare_dram_parameter(
                f"{nmm}{t}", [PAD, D], f32, isOutput=False)
    wnode = nc.declare_dram_parameter("wnode", [L, 3, D, D], bf16, isOutput=False)
    wrn = nc.declare_dram_parameter("wrn", [L, 2, D, D], bf16, isOutput=False)
    biasrep = nc.declare_dram_parameter("biasrep", [L, 2, P, D], f32, isOutput=False)
    blf = nc.declare_dram_parameter("blf", [D, 2], f32, isOutput=False)
    recipf = nc.declare_dram_parameter("recipf", [3, D, PAD], bf16, isOutput=False)
    masters = [[nc.declare_dram_parameter(f"m{e}_{l}", [PAD, P], bf16,
                                          isOutput=False)
                for e in range(3)] for l in range(L)]
    gidx = nc.declare_dram_parameter("gidx", [128, cbtot], i16, isOutput=False)
    sidx = nc.declare_dram_parameter("sidx", [3, NR, 128, PAD // 16], i16,
                                     isOutput=False)
    outs = [nc.declare_dram_parameter(f"out{t}", [PAD, D], f32, isOutput=True)
            for t in "AB"]

    # ---------------- DRAM internals
    stg = [nc.dram_tensor(f"stg{t}", [PAD, D], f32) for t in "AB"]
    tf1 = [nc.dram_tensor(f"tf1{t}", [D, PAD], bf16) for t in "AB"]
    tab_space = "Shared" if NCO > 4 else "Local"
    tab1 = [nc.dram_tensor(f"tab1{t}", [TROWS, D], f32, addr_space=tab_space)
            for t in "AB"]

    # ---------------- width registers (before TileContext)
    wregs = {}
    for w in WSET:
        r = nc.alloc_register(mybir.EngineType.Pool, f"w{w}")
        nc.gpsimd.reg_mov(r, w)
        wregs[w] = r

    rearr = "(p g) d -> p g d"

    with tile.TileContext(nc) as tc:
        with tc.tile_pool(name="const", bufs=1) as cpool, \
             tc.tile_pool(name="accp", bufs=1) as apool, \
             tc.tile_pool(name="flushp", bufs=1) as fpool, \
             tc.tile_pool(name="idxp", bufs=16) as ipool, \
             tc.tile_pool(name="sidxp", bufs=2) as spool, \
             tc.tile_pool(name="msgp", bufs=4) as mpool, \
             tc.tile_pool(name="aggfp", bufs=1) as gpool, \
             tc.tile_pool(name="recfp", bufs=2) as rpool, \
             tc.tile_pool(name="densep", bufs=2) as dpool, \
             tc.tile_pool(name="psA", bufs=2, space="PSUM") as psA, \
             tc.tile_pool(name="psB", bufs=2, space="PSUM") as psB, \
             tc.tile_pool(name="psFA", bufs=2, space="PSUM") as psFA, \
             tc.tile_pool(name="psFB", bufs=2, space="PSUM") as psFB:

            # ---- constants
            wn_t = cpool.tile([D, L * 3, D], bf16)
            nc.sync.dma_start(out=wn_t[:], in_=wnode[:].rearrange("l e a b -> a (l e) b"))
            wr_t = cpool.tile([D, L * 2, D], bf16)
            nc.sync.dma_start(out=wr_t[:], in_=wrn[:].rearrange("l t a b -> a (l t) b"))
            brep_t = cpool.tile([P, L * 2, D], f32)
            nc.sync.dma_start(out=brep_t[:], in_=biasrep[:].rearrange("l t p d -> p (l t) d"))
            blf_t = cpool.tile([D, 2], f32)
            nc.sync.dma_start(out=blf_t[:], in_=blf[:])

            zero_small = cpool.tile([P, D], f32)
            nc.vector.memset(zero_small[:], 0.0)

            # flush buffer (bf16, 256B rows); zero once so pad cols stay clean
            flush_t = fpool.tile([P, G, P], bf16, tag="flush", name="flush")
            nc.vector.memset(flush_t[:], 0.0)

            # ============ per layer ============
            for l in range(L):
                tabs = [tabx[0], tabx[1]] if l == 0 else [tab1[0], tab1[1]]
                last = (l == L - 1)

                accs = [apool.tile([P, G, D], f32, tag=f"acc{e}", name=f"acc_{l}_{e}")
                        for e in range(3)]
                cur_r = [0, 0, 0]
                for e in range(3):
                    nc.vector.memset(accs[e][:], 0.0)

                def flush(e, r, accs=accs, l=l):
                    # cast+copy the slot accumulator to bf16 (frees acc),
                    # then 4 scatter-add chunks into the master
                    fb = fpool.tile([P, G, P], bf16, tag="flush",
                                    name=f"fl_{l}_{e}_{r}")
                    nc.vector.tensor_copy(out=fb[:, :, 0:D], in_=accs[e][:])
                    sx = spool.tile([P, PAD // 16], i16, tag="sx",
                                    name=f"sx_{l}_{e}_{r}")
                    nc.sync.dma_start(out=sx[:], in_=sidx[e, r])
                    if skip_scatter:
                        return
                    for (g0, g1) in cfg["scatter_chunks"]:
                        w = (g1 - g0) * P
                        nc.gpsimd.dma_scatter_add(
                            masters[l][e][:], fb[:, g0:g1, :],
                            sx[:, g0 * 8:g1 * 8], w, wregs[w], P,
                            single_packet=True, queue_num=0)

                col = 0
                qn = 0
                for (e, r, k, go, wp) in schedule:
                    if r != cur_r[e]:
                        flush(e, cur_r[e])
                        cur_r[e] = r
                        accs[e] = apool.tile([P, G, D], f32, tag=f"acc{e}",
                                             name=f"acc_{l}_{e}_r{r}")
                        nc.vector.memset(accs[e][:], 0.0)
                    cb = wp // 16
                    gw = wp // P
                    idx_t = ipool.tile([P, cb], i16, tag="gi", name=f"gi_{l}_{qn}")
                    nc.sync.dma_start(out=idx_t[:], in_=gidx[:, col:col + cb])
                    msg = mpool.tile([P, gw, D], f32, tag="msg", name=f"msg_{l}_{qn}")
                    sT = ETS[e][1]
                    if not skip_gather:
                        nc.gpsimd.dma_gather(
                            out_ap=msg[:],
                            in_ap=tabs[sT][r * RWIN:(r + 1) * RWIN, :],
                            idxs_ap=idx_t[:],
                            num_idxs=wp, num_idxs_reg=wregs[wp], elem_size=D,
                            single_packet=False, queue_num=qn % 4)
                        nc.vector.tensor_add(out=accs[e][:, go:go + gw, :],
                                             in0=accs[e][:, go:go + gw, :], in1=msg[:])
                    qn += 1
                    col += cb
                for e in range(3):
                    flush(e, cur_r[e])

                # ---- dense stage: feature-major, transpose-free
                if skip_dense:
                    continue
                for h in range(2):
                    rows0 = h * HR
                    aggf = []
                    for e in range(3):
                        af = gpool.tile([P, HR], bf16, tag=f"agf{e}",
                                        name=f"agf_{l}_{h}_{e}")
                        nc.sync.dma_start(out=af[:],
                                          in_=masters[l][e][rows0:rows0 + HR, :],
                                          transpose=True)
                        rf = rpool.tile([D, HR], bf16, tag="rcf",
                                        name=f"rcf_{l}_{h}_{e}")
                        nc.sync.dma_start(out=rf[:],
                                          in_=recipf[e, :, rows0:rows0 + HR])
                        nc.vector.tensor_mul(out=af[0:D, :], in0=af[0:D, :], in1=rf[:])
                        aggf.append(af)

                    nblk = (HG + BL - 1) // BL
                    for b in range(nblk):
                        gw = min(BL, HG - b * BL)
                        c0 = b * BL * P           # col offset within half
                        cw = gw * P
                        gcol = rows0 + c0          # global node col offset
                        # t_dst chunks (feature-major, from DRAM)
                        tfd = [xf[0], xf[1]] if l == 0 else [tf1[0], tf1[1]]
                        tfa = dpool.tile([D, BL * P], bf16, tag="tfa", name=f"tfa_{l}_{h}_{b}")
                        tfb = dpool.tile([D, BL * P], bf16, tag="tfb", name=f"tfb_{l}_{h}_{b}")
                        nc.sync.dma_start(out=tfa[:, 0:cw], in_=tfd[0][:, gcol:gcol + cw])
                        nc.sync.dma_start(out=tfb[:, 0:cw], in_=tfd[1][:, gcol:gcol + cw])

                        tn = dpool.tile([P, BL, D], f32, tag="tn", name=f"tn_{l}_{h}_{b}")
                        for gi in range(gw):
                            lc = c0 + gi * P
                            # type A node-major: aggBA@W + aggAA@W + tfa@Wr
                            pA = psA.tile([P, D], f32, tag="pa", name=f"pA_{l}_{h}_{b}_{gi}")
                            nc.tensor.matmul(out=pA[:], lhsT=aggf[1][0:D, lc:lc + P],
                                             rhs=wn_t[:, l * 3 + 1, :], start=True, stop=False)
                            nc.tensor.matmul(out=pA[:], lhsT=aggf[2][0:D, lc:lc + P],
                                             rhs=wn_t[:, l * 3 + 2, :], start=False, stop=False)
                            nc.tensor.matmul(out=pA[:], lhsT=tfa[:, gi * P:(gi + 1) * P],
                                             rhs=wr_t[:, l * 2 + 0, :], start=False, stop=True)
                            # type B node-major: aggAB@W + tfb@Wr
                            pB = psB.tile([P, D], f32, tag="pb", name=f"pB_{l}_{h}_{b}_{gi}")
                            nc.tensor.matmul(out=pB[:], lhsT=aggf[0][0:D, lc:lc + P],
                                             rhs=wn_t[:, l * 3 + 0, :], start=True, stop=False)
                            nc.tensor.matmul(out=pB[:], lhsT=tfb[:, gi * P:(gi + 1) * P],
                                             rhs=wr_t[:, l * 2 + 1, :], start=False, stop=True)
                            for t, ps in ((0, pA), (1, pB)):
                                nc.vector.tensor_add(out=tn[:, gi, :], in0=ps[:],
                                                     in1=brep_t[:, l * 2 + t, :])
                                if last:
                                    nc.scalar.activation(
                                        tn[:, gi, :], tn[:, gi, :],
                                        mybir.ActivationFunctionType.Tanh)
                                nc.sync.dma_start(
                                    out=stg[t][gcol + gi * P:gcol + (gi + 1) * P, :],
                                    in_=tn[:, gi, :])
                        # feature-major next-layer t (layer 0 only)
                        if not last:
                            for t, agl, tfx in ((0, (1, 2), tfa), (1, (0,), tfb)):
                                pF = (psFA if t == 0 else psFB).tile(
                                    [D, BL * P], f32, tag="pf", name=f"pF_{l}_{h}_{b}_{t}")
                                first = True
                                for e in agl:
                                    nc.tensor.matmul(out=pF[:, 0:cw],
                                                     lhsT=wn_t[:, l * 3 + e, :],
                                                     rhs=aggf[e][0:D, c0:c0 + cw],
                                                     start=first, stop=False)
                                    first = False
                                nc.tensor.matmul(out=pF[:, 0:cw],
                                                 lhsT=wr_t[:, l * 2 + t, :],
                                                 rhs=tfx[:, 0:cw],
                                                 start=False, stop=True)
                                tfo = dpool.tile([D, BL * P], bf16, tag=f"tfo{t}",
                                                 name=f"tfo_{l}_{h}_{b}_{t}")
                                nc.scalar.activation(
                                    tfo[:, 0:cw], pF[:, 0:cw],
                                    mybir.ActivationFunctionType.Identity,
                                    bias=blf_t[:, t:t + 1])
                                nc.sync.dma_start(out=tf1[t][:, gcol:gcol + cw],
                                                  in_=tfo[:, 0:cw])

                if not last:
                    for t in range(2):
                        if PAD > SH:
                            nc.sync.dma_start(out=stg[t][SH:PAD, :],
                                              in_=zero_small[0:PAD - SH, :])
                        if skip_cc:
                            nc.sync.dma_start(out=tab1[t][0:PAD, :], in_=stg[t][:])
                        else:
                            nc.gpsimd.collective_compute(
                                "AllGather", mybir.AluOpType.bypass,
                                replica_groups=[list(range(NCO))],
                                ins=[stg[t][:]], outs=[tab1[t][:]])

            # ---- gates: out = f*c + i*tanh_t   (tanh_t staged in stg)
            ngc = (G + GCHUNK - 1) // GCHUNK
            for t in range(2):
                tname = "AB"[t]
                for b in range(ngc):
                    g0 = b * GCHUNK
                    g1 = min(g0 + GCHUNK, G)
                    gw = g1 - g0
                    r0, r1 = g0 * P, g1 * P
                    ct = mpool.tile([P, WCAP_G, D], f32, tag="msg", name=f"ct{t}_{b}")
                    it = mpool.tile([P, WCAP_G, D], f32, tag="msg", name=f"it{t}_{b}")
                    ft = mpool.tile([P, WCAP_G, D], f32, tag="msg", name=f"ft{t}_{b}")
                    tt = mpool.tile([P, WCAP_G, D], f32, tag="msg", name=f"tt{t}_{b}")
                    nc.sync.dma_start(out=ct[:, 0:gw, :],
                                      in_=gates_in["c" + tname][r0:r1, :].rearrange(rearr, p=P))
                    nc.sync.dma_start(out=it[:, 0:gw, :],
                                      in_=gates_in["i" + tname][r0:r1, :].rearrange(rearr, p=P))
                    nc.sync.dma_start(out=ft[:, 0:gw, :],
                                      in_=gates_in["f" + tname][r0:r1, :].rearrange(rearr, p=P))
                    nc.sync.dma_start(out=tt[:, 0:gw, :],
                                      in_=stg[t][r0:r1, :].rearrange(rearr, p=P))
                    nc.vector.tensor_mul(out=ft[:, 0:gw, :], in0=ft[:, 0:gw, :], in1=ct[:, 0:gw, :])
                    nc.vector.tensor_mul(out=it[:, 0:gw, :], in0=it[:, 0:gw, :], in1=tt[:, 0:gw, :])
                    nc.vector.tensor_add(out=ft[:, 0:gw, :], in0=ft[:, 0:gw, :], in1=it[:, 0:gw, :])
                    nc.sync.dma_start(out=outs[t][r0:r1, :].rearrange(rearr, p=P),
                                      in_=ft[:, 0:gw, :])

    # Align SWDGE queue_num with Tile's DMASW semaphore lane assignment:
    # each DMASW sem must only ever be updated from one SWDGE queue, and
    # Tile assigns lanes round-robin over the scheduled order. queue = lane%4.
    import re as _re
    qcount = {}
    for _ins in list(nc.inst_map.values()):
        if isinstance(_ins, (mybir.InstDMAGatherAnt, mybir.InstDMAScatterAddAnt)):
            _si = _ins.sync_info
            for _u in (_si.on_update or []):
                _m = _re.match(r"DMASW(\d+)", getattr(_u, "ant_name", "") or "")
                if _m:
                    _ins.queue_num = int(_m.group(1)) % 4
                    qcount[_ins.queue_num] = qcount.get(_ins.queue_num, 0) + 1
                    break
    nc._swdge_queue_dist = qcount

    nc.compile()
    return nc


# ---------------------------------------------------------------- host wrapper

def make_in_maps(cfg, inputs, percore):
    import ml_dtypes
    bf16 = ml_dtypes.bfloat16
    cfg = cfg_derived(cfg)
    NCO, SH, PAD, TROWS = cfg["n_cores"], cfg["shard"], cfg["pad"], cfg["trows"]

    def pad_rows(a):
        out = np.zeros((PAD, D), np.float32)
        out[:SH] = a
        return out

    # full x in table layout
    tabx = {}
    for t, xn in (("A", "x_A"), ("B", "x_B")):
        tb = np.zeros((TROWS, D), np.float32)
        x = np.asarray(inputs[xn], np.float32)
        for c in range(NCO):
            tb[PAD * c:PAD * c + SH] = x[SH * c:SH * (c + 1)]
        tabx[t] = tb

    Wl = np.asarray(inputs["Wl"], np.float32)
    Wr = np.asarray(inputs["Wr"], np.float32)
    bl = np.asarray(inputs["bl"], np.float32)
    WxA = np.asarray(inputs["Wx_A"], np.float32)
    WxB = np.asarray(inputs["Wx_B"], np.float32)
    biasA = np.asarray(inputs["bias_A"], np.float32)
    biasB = np.asarray(inputs["bias_B"], np.float32)
    Wx = [WxA, WxB]

    # wnode[l, e] = lhs-folded (Wl[l,e] @ Wx_src)^T for l==0 else Wl[l,e]^T
    wnode = np.zeros((L, 3, D, D), np.float32)
    for l in range(L):
        for e, (_, sT, dT) in enumerate(ETS):
            w = Wl[l, e] @ Wx[sT] if l == 0 else Wl[l, e]
            wnode[l, e] = w.T
    # wrn[l, t]: dst-type folded Wr sums
    wrn = np.zeros((L, 2, D, D), np.float32)
    for l in range(L):
        wrA = Wr[l, 1] + Wr[l, 2]
        wrB = Wr[l, 0]
        if l == 0:
            wrA = wrA @ WxA
            wrB = wrB @ WxB
        wrn[l, 0] = wrA.T
        wrn[l, 1] = wrB.T

    # biases: layer bl sums per dst type (+ final per-type bias at last layer)
    bl_t = np.zeros((L, 2, D), np.float32)
    for l in range(L):
        bl_t[l, 0] = bl[l, 1] + bl[l, 2]
        bl_t[l, 1] = bl[l, 0]
    bl_t[L - 1, 0] += biasA
    bl_t[L - 1, 1] += biasB
    biasrep = np.broadcast_to(bl_t[:, :, None, :], (L, 2, P, D)).copy()
    blf = np.ascontiguousarray(bl_t[0].T)  # [D, 2] (layer-0 feat-major bias)

    mz = np.zeros((PAD, P), bf16)

    in_maps = []
    for c in range(NCO):
        sl = slice(SH * c, SH * (c + 1))
        deg = percore[c]["deg"]  # [3, PAD] int32
        recipf = np.zeros((3, D, PAD), np.float32)
        recipf[:, :, :] = (1.0 / np.maximum(deg, 1.0))[:, None, :]
        xf = {}
        for t, xn in (("A", "x_A"), ("B", "x_B")):
            a = np.zeros((D, PAD), np.float32)
            a[:, :SH] = np.asarray(inputs[xn], np.float32)[sl].T
            xf[t] = a.astype(bf16)
        m = {
            "tabxA": tabx["A"], "tabxB": tabx["B"],
            "xfA": xf["A"], "xfB": xf["B"],
            "wnode": wnode.astype(bf16), "wrn": wrn.astype(bf16),
            "biasrep": biasrep, "blf": blf,
            "recipf": recipf.astype(bf16),
            "gidx": percore[c]["gidx"],
            "sidx": percore[c]["sidx"],
        }
        for l in range(L):
            for e in range(3):
                m[f"m{e}_{l}"] = mz
        for t in "AB":
            for nmm in "cif":
                m[f"{nmm}{t}"] = pad_rows(np.asarray(inputs[f"{nmm}_{t}"])[sl])
        in_maps.append(m)
    return in_maps


_BUILT = {}


def kernel(**inputs):
    from concourse.bass_utils import run_bass_kernel_spmd

    cfg0 = full_cfg()
    edges = {"AB": np.asarray(inputs["edge_AB"]),
             "BA": np.asarray(inputs["edge_BA"]),
             "AA": np.asarray(inputs["edge_AA"])}
    cfg, schedule, percore, cbtot = host_prep(cfg0, edges)

    key = (cbtot, tuple(schedule))
    if key not in _BUILT:
        _BUILT.clear()
        _BUILT[key] = build(cfg0, schedule, cbtot)
    nc = _BUILT[key]

    in_maps = make_in_maps(cfg0, inputs, percore)
    r = run_bass_kernel_spmd(nc, in_maps, core_ids=list(range(cfg["n_cores"])))

    SH = cfg["shard"]
    out_A = np.concatenate([r.results[c]["outA"][:SH] for c in range(cfg["n_cores"])], axis=0)
    out_B = np.concatenate([r.results[c]["outB"][:SH] for c in range(cfg["n_cores"])], axis=0)
    return (out_A, out_B)


# revision 19
# speedup vs baseline: 1.5612x; 1.3249x over previous
"""Trainium2 Bass kernel for nn_CellGate (hetero GNN message passing + LSTM-style gate).

Strategy (8-core SPMD, dst-sharded), v2:
- Each core owns a contiguous 12,500-node shard of both node types (A and B).
- Segment-mean aggregation per edge type via `dma_gather` slot passes (as v1),
  but with small pass widths (<=24 groups), deep idx/msg pools and round-robin
  SWDGE queues so all four Q7 descriptor-generation pairs run concurrently.
- Per-(et,range) flush: one DVE cast f32->bf16 of the slot accumulator, then 4
  `dma_scatter_add`s (bf16, 256B rows) into a host-zeroed DRAM master.
- Dense stage is transpose-free: masters are read back with HWDGE DMA-transpose
  into feature-major bf16 [64, nodes] tiles, recip (mean) applied there once,
  and per-128-node-chunk matmuls consume the same feature-major tile as lhsT
  (node-major output) and as rhs (feature-major output for the next layer's
  W_r term). Biases ride in via partition-replicated adds / activation bias.
- Weights (incl. the Wx input-projection folds for layer 0) are folded on host
  and shipped bf16.
- One AllGather per node type rebuilds the full f32 gather table between
  layers; gates are elementwise on DVE at the end.
"""

import numpy as np

import concourse.bass as bass
import concourse.bacc as bacc
import concourse.mybir as mybir
import concourse.tile as tile

P = 128
D = 64

# edge types: (name, src_type, dst_type)
ETS = [("AB", 0, 1), ("BA", 1, 0), ("AA", 0, 0)]
L = 2

WCAP_G = 24              # max gather pass width in groups of 128
WSET_G = list(range(1, 26))   # width register values (multiples of 128)


def full_cfg():
    return dict(n_cores=8, shard=12500, G=98, n_ranges=4, spr=2)


def cfg_derived(cfg):
    c = dict(cfg)
    c["pad"] = P * c["G"]
    c["rwin"] = c["spr"] * c["pad"]
    c["trows"] = c["n_cores"] * c["pad"]
    c["nnodes"] = c["n_cores"] * c["shard"]
    c["wset"] = [g * P for g in WSET_G]
    # scatter chunk group ranges (4 chunks per flush)
    sch = []
    g0 = 0
    while g0 < c["G"]:
        g1 = min(g0 + 25, c["G"])
        sch.append((g0, g1))
        g0 = g1
    c["scatter_chunks"] = sch
    return c


def roundw(w, wset):
    for v in wset:
        if v >= w:
            return v
    return wset[-1]


# ---------------------------------------------------------------- host prep

def host_prep(cfg, edges):
    """edges: dict name -> [2, E] int32 (src, dst global).

    Returns: schedule (static, shared): list of passes (et_i, r, k, go, wp)
    and per-core arrays:
      gidx[core]: int16 [128, CBTOT]  (packed gather indices, 8-replicated)
      sidx[core]: int16 [3, n_ranges, 128, pad//16]
      deg[core]:  int32 [3, pad]   (total in-degree per et)
    """
    cfg = cfg_derived(cfg)
    NCO, SH, PAD, G = cfg["n_cores"], cfg["shard"], cfg["pad"], cfg["G"]
    NR, RWIN, WSET = cfg["n_ranges"], cfg["rwin"], cfg["wset"]

    percore = [dict(sidx=np.zeros((3, NR, 128, PAD // 16), np.int16),
                    deg=np.zeros((3, PAD), np.int32)) for _ in range(NCO)]

    all_counts = {}   # (et_i, r, k) -> max over cores of count
    maxk = {}         # (et_i, r) -> K
    core_data = {}    # (core, et_i, r) -> (theta, by_k list)
    for et_i, (etn, sT, dT) in enumerate(ETS):
        src, dst = edges[etn][0].astype(np.int64), edges[etn][1].astype(np.int64)
        srow = PAD * (src // SH) + (src - SH * (src // SH))  # global table row
        for c in range(NCO):
            m = (dst // SH) == c
            s_r, d_l = srow[m], dst[m] - c * SH
            percore[c]["deg"][et_i] = np.bincount(d_l, minlength=PAD)[:PAD]
            for r in range(NR):
                rm = (s_r // RWIN) == r
                sl, dl = s_r[rm] - r * RWIN, d_l[rm]
                deg_r = np.bincount(dl, minlength=PAD)[:PAD]
                theta = np.argsort(-deg_r, kind="stable")
                slot_of = np.empty(PAD, np.int64)
                slot_of[theta] = np.arange(PAD)
                K = int(deg_r.max()) if deg_r.size else 0
                maxk[(et_i, r)] = max(maxk.get((et_i, r), 0), K)
                order = np.argsort(dl, kind="stable")
                dls, sls = dl[order], sl[order]
                if dls.size:
                    starts = np.r_[0, np.nonzero(np.diff(dls))[0] + 1]
                    rank = np.arange(dls.size) - np.repeat(
                        starts, np.diff(np.r_[starts, dls.size]))
                else:
                    rank = np.zeros(0, np.int64)
                by_k = []
                for k in range(K):
                    km = rank == k
                    dk, sk = dls[km], sls[km]
                    all_counts[(et_i, r, k)] = max(
                        all_counts.get((et_i, r, k), 0), dk.size)
                    by_k.append((slot_of[dk], sk))
                core_data[(c, et_i, r)] = (theta, by_k)
                w = theta.astype(np.int16).reshape(PAD // 16, 16).T
                percore[c]["sidx"][et_i, r] = np.tile(w, (8, 1))

    # static schedule, pieces of <= WCAP_G groups, round-robin across ets
    schedule = []
    for et_i in range(3):
        for r in range(NR):
            for k in range(maxk.get((et_i, r), 0)):
                w = ((max(all_counts.get((et_i, r, k), 1), 1) + P - 1) // P) * P
                go = 0
                while go * P < w:
                    gw = min(WCAP_G, w // P - go)
                    wp = roundw(gw * P, WSET)
                    schedule.append((et_i, r, k, go, wp))
                    go += wp // P
    per_et = [[] for _ in range(3)]
    for t in sorted(schedule, key=lambda t: (t[0], t[1], t[2], t[3])):
        per_et[t[0]].append(t)
    schedule = []
    i = [0, 0, 0]
    while any(i[e] < len(per_et[e]) for e in range(3)):
        for e in range(3):
            if i[e] < len(per_et[e]):
                schedule.append(per_et[e][i[e]])
                i[e] += 1

    # packed gather idx per core
    ZLOC = SH  # zero-row local index within each range window
    for c in range(NCO):
        full_arr = {}
        for (et_i, r, k, go, wp) in schedule:
            key = (c, et_i, r, k)
            if key not in full_arr:
                theta, by_k = core_data[(c, et_i, r)]
                arr = np.full(PAD, ZLOC, np.int16)
                if k < len(by_k):
                    slots, srcs = by_k[k]
                    arr[slots] = srcs.astype(np.int16)
                full_arr[key] = arr
        parts = []
        for (et_i, r, k, go, wp) in schedule:
            arr = np.full(wp, ZLOC, np.int16)
            seg = full_arr[(c, et_i, r, k)][go * P:go * P + wp]
            arr[:seg.size] = seg
            wrapped = arr.reshape(wp // 16, 16).T
            parts.append(np.tile(wrapped, (8, 1)))
        percore[c]["gidx"] = np.concatenate(parts, axis=1)
    cbtot = percore[0]["gidx"].shape[1]
    return cfg, schedule, percore, cbtot


# ---------------------------------------------------------------- builder

def build(cfg, schedule, cbtot, skip_gather=False, skip_dense=False,
          skip_cc=False, skip_scatter=False):
    cfg = cfg_derived(cfg)
    NCO, SH, PAD, G = cfg["n_cores"], cfg["shard"], cfg["pad"], cfg["G"]
    NR, RWIN, TROWS, WSET = cfg["n_ranges"], cfg["rwin"], cfg["trows"], cfg["wset"]
    f32 = mybir.dt.float32
    bf16 = mybir.dt.bfloat16
    i16 = mybir.dt.int16
    HG = 49                    # groups per dense half
    HR = HG * P                # rows per dense half
    BL = 4                     # dense groups per block
    GCHUNK = 20                # gates groups per chunk

    nc = bacc.Bacc(None, target_bir_lowering=False, debug=False,
                   num_swdge_queues=4, num_devices=NCO)

    # ---------------- inputs
    tabx = [nc.declare_dram_parameter(f"tabx{t}", [TROWS, D], f32, isOutput=False)
            for t in "AB"]
    xf = [nc.declare_dram_parameter(f"xf{t}", [D, PAD], bf16, isOutput=False)
          for t in "AB"]
    gates_in = {}
    for t in "AB":
        for nmm in "cif":
            gates_in[nmm + t] = nc.declare_dram_parameter(
                f"{nmm}{t}", [PAD, D], f32, isOutput=False)
    wnode = nc.declare_dram_parameter("wnode", [L, 3, D, D], bf16, isOutput=False)
    wrn = nc.declare_dram_parameter("wrn", [L, 2, D, D], bf16, isOutput=False)
    biasrep = nc.declare_dram_parameter("biasrep", [L, 2, P, D], f32, isOutput=False)
    blf = nc.declare_dram_parameter("blf", [D, 2], f32, isOutput=False)
    recipf = nc.declare_dram_parameter("recipf", [3, D, PAD], bf16, isOutput=False)
    masters = [[nc.declare_dram_parameter(f"m{e}_{l}", [PAD, P], bf16,
                                          isOutput=False)
                for e in range(3)] for l in range(L)]
    gidx = nc.declare_dram_parameter("gidx", [128, cbtot], i16, isOutput=False)
    sidx = nc.declare_dram_parameter("sidx", [3, NR, 128, PAD // 16], i16,
                                     isOutput=False)
    outs = [nc.declare_dram_parameter(f"out{t}", [PAD, D], f32, isOutput=True)
            for t in "AB"]

    # ---------------- DRAM internals
    stg = [nc.dram_tensor(f"stg{t}", [PAD, D], f32) for t in "AB"]
    tf1 = [nc.dram_tensor(f"tf1{t}", [D, PAD], bf16) for t in "AB"]
    tab_space = "Shared" if NCO > 4 else "Local"
    tab1 = [nc.dram_tensor(f"tab1{t}", [TROWS, D], f32, addr_space=tab_space)
            for t in "AB"]

    # ---------------- width registers (before TileContext)
    wregs = {}
    for w in WSET:
        r = nc.alloc_register(mybir.EngineType.Pool, f"w{w}")
        nc.gpsimd.reg_mov(r, w)
        wregs[w] = r

    rearr = "(p g) d -> p g d"

    with tile.TileContext(nc) as tc:
        with tc.tile_pool(name="const", bufs=1) as cpool, \
             tc.tile_pool(name="accp", bufs=1) as apool, \
             tc.tile_pool(name="idxp", bufs=16) as ipool, \
             tc.tile_pool(name="sidxp", bufs=2) as spool, \
             tc.tile_pool(name="msgp", bufs=8) as mpool, \
             tc.tile_pool(name="aggfp", bufs=1) as gpool, \
             tc.tile_pool(name="recfp", bufs=2) as rpool, \
             tc.tile_pool(name="densep", bufs=3) as dpool, \
             tc.tile_pool(name="psA", bufs=2, space="PSUM") as psA, \
             tc.tile_pool(name="psB", bufs=2, space="PSUM") as psB, \
             tc.tile_pool(name="psFA", bufs=2, space="PSUM") as psFA, \
             tc.tile_pool(name="psFB", bufs=2, space="PSUM") as psFB:

            # ---- constants
            wn_t = cpool.tile([D, L * 3, D], bf16)
            nc.sync.dma_start(out=wn_t[:], in_=wnode[:].rearrange("l e a b -> a (l e) b"))
            wr_t = cpool.tile([D, L * 2, D], bf16)
            nc.sync.dma_start(out=wr_t[:], in_=wrn[:].rearrange("l t a b -> a (l t) b"))
            brep_t = cpool.tile([P, L * 2, D], f32)
            nc.sync.dma_start(out=brep_t[:], in_=biasrep[:].rearrange("l t p d -> p (l t) d"))
            blf_t = cpool.tile([D, 2], f32)
            nc.sync.dma_start(out=blf_t[:], in_=blf[:])

            zero_small = cpool.tile([P, D], f32)
            nc.vector.memset(zero_small[:], 0.0)

            # ============ per layer ============
            for l in range(L):
                tabs = [tabx[0], tabx[1]] if l == 0 else [tab1[0], tab1[1]]
                last = (l == L - 1)

                accs = [apool.tile([P, G, D], bf16, tag=f"acc{e}", name=f"acc_{l}_{e}")
                        for e in range(3)]
                cur_r = [0, 0, 0]
                for e in range(3):
                    nc.vector.memset(accs[e][:], 0.0)

                def flush(e, r, accs=accs, l=l):
                    # scatter-add the bf16 slot accumulator into the master
                    sx = spool.tile([P, PAD // 16], i16, tag="sx",
                                    name=f"sx_{l}_{e}_{r}")
                    nc.sync.dma_start(out=sx[:], in_=sidx[e, r])
                    if skip_scatter:
                        return
                    for (g0, g1) in cfg["scatter_chunks"]:
                        w = (g1 - g0) * P
                        nc.gpsimd.dma_scatter_add(
                            masters[l][e][:, 0:D], accs[e][:, g0:g1, :],
                            sx[:, g0 * 8:g1 * 8], w, wregs[w], D,
                            elem_step=P,
                            single_packet=False, queue_num=0)

                col = 0
                qn = 0
                for (e, r, k, go, wp) in schedule:
                    if r != cur_r[e]:
                        flush(e, cur_r[e])
                        cur_r[e] = r
                        accs[e] = apool.tile([P, G, D], bf16, tag=f"acc{e}",
                                             name=f"acc_{l}_{e}_r{r}")
                        nc.vector.memset(accs[e][:], 0.0)
                    cb = wp // 16
                    gw = wp // P
                    idx_t = ipool.tile([P, cb], i16, tag="gi", name=f"gi_{l}_{qn}")
                    nc.sync.dma_start(out=idx_t[:], in_=gidx[:, col:col + cb])
                    msg = mpool.tile([P, gw, D], f32, tag="msg", name=f"msg_{l}_{qn}")
                    sT = ETS[e][1]
                    if not skip_gather:
                        nc.gpsimd.dma_gather(
                            out_ap=msg[:],
                            in_ap=tabs[sT][r * RWIN:(r + 1) * RWIN, :],
                            idxs_ap=idx_t[:],
                            num_idxs=wp, num_idxs_reg=wregs[wp], elem_size=D,
                            single_packet=False, queue_num=qn % 4)
                        nc.vector.tensor_add(out=accs[e][:, go:go + gw, :],
                                             in0=accs[e][:, go:go + gw, :], in1=msg[:])
                    qn += 1
                    col += cb
                for e in range(3):
                    flush(e, cur_r[e])

                # ---- dense stage: feature-major, transpose-free
                if skip_dense:
                    continue
                for h in range(2):
                    rows0 = h * HR
                    aggf = []
                    for e in range(3):
                        af = gpool.tile([P, HR], bf16, tag=f"agf{e}",
                                        name=f"agf_{l}_{h}_{e}")
                        nc.sync.dma_start(out=af[:],
                                          in_=masters[l][e][rows0:rows0 + HR, :],
                                          transpose=True)
                        rf = rpool.tile([D, HR], bf16, tag="rcf",
                                        name=f"rcf_{l}_{h}_{e}")
                        nc.sync.dma_start(out=rf[:],
                                          in_=recipf[e, :, rows0:rows0 + HR])
                        nc.vector.tensor_mul(out=af[0:D, :], in0=af[0:D, :], in1=rf[:])
                        aggf.append(af)

                    nblk = (HG + BL - 1) // BL
                    for b in range(nblk):
                        gw = min(BL, HG - b * BL)
                        c0 = b * BL * P           # col offset within half
                        cw = gw * P
                        gcol = rows0 + c0          # global node col offset
                        # t_dst chunks (feature-major, from DRAM)
                        tfd = [xf[0], xf[1]] if l == 0 else [tf1[0], tf1[1]]
                        tfa = dpool.tile([D, BL * P], bf16, tag="tfa", name=f"tfa_{l}_{h}_{b}")
                        tfb = dpool.tile([D, BL * P], bf16, tag="tfb", name=f"tfb_{l}_{h}_{b}")
                        nc.sync.dma_start(out=tfa[:, 0:cw], in_=tfd[0][:, gcol:gcol + cw])
                        nc.sync.dma_start(out=tfb[:, 0:cw], in_=tfd[1][:, gcol:gcol + cw])

                        tns = [dpool.tile([P, BL, D], f32, tag=f"tn{t}",
                                          name=f"tn{t}_{l}_{h}_{b}") for t in range(2)]
                        for gi in range(gw):
                            lc = c0 + gi * P
                            # type A node-major: aggBA@W + aggAA@W + tfa@Wr
                            pA = psA.tile([P, D], f32, tag="pa", name=f"pA_{l}_{h}_{b}_{gi}")
                            nc.tensor.matmul(out=pA[:], lhsT=aggf[1][0:D, lc:lc + P],
                                             rhs=wn_t[:, l * 3 + 1, :], start=True, stop=False)
                            nc.tensor.matmul(out=pA[:], lhsT=aggf[2][0:D, lc:lc + P],
                                             rhs=wn_t[:, l * 3 + 2, :], start=False, stop=False)
                            nc.tensor.matmul(out=pA[:], lhsT=tfa[:, gi * P:(gi + 1) * P],
                                             rhs=wr_t[:, l * 2 + 0, :], start=False, stop=True)
                            # type B node-major: aggAB@W + tfb@Wr
                            pB = psB.tile([P, D], f32, tag="pb", name=f"pB_{l}_{h}_{b}_{gi}")
                            nc.tensor.matmul(out=pB[:], lhsT=aggf[0][0:D, lc:lc + P],
                                             rhs=wn_t[:, l * 3 + 0, :], start=True, stop=False)
                            nc.tensor.matmul(out=pB[:], lhsT=tfb[:, gi * P:(gi + 1) * P],
                                             rhs=wr_t[:, l * 2 + 1, :], start=False, stop=True)
                            for t, ps in ((0, pA), (1, pB)):
                                nc.vector.tensor_add(out=tns[t][:, gi, :], in0=ps[:],
                                                     in1=brep_t[:, l * 2 + t, :])
                                if last:
                                    nc.scalar.activation(
                                        tns[t][:, gi, :], tns[t][:, gi, :],
                                        mybir.ActivationFunctionType.Tanh)
                        brearr = "(g p) d -> p g d"
                        if not last:
                            for t in range(2):
                                nc.sync.dma_start(
                                    out=stg[t][gcol:gcol + cw, :].rearrange(brearr, p=P),
                                    in_=tns[t][:, 0:gw, :])
                        else:
                            # fused gates: out = f*c + i*tanh_t
                            for t in range(2):
                                tname = "AB"[t]
                                ct = mpool.tile([P, WCAP_G, D], f32, tag="msg",
                                                name=f"ct{t}_{h}_{b}")
                                it = mpool.tile([P, WCAP_G, D], f32, tag="msg",
                                                name=f"it{t}_{h}_{b}")
                                ftl = mpool.tile([P, WCAP_G, D], f32, tag="msg",
                                                 name=f"ft{t}_{h}_{b}")
                                nc.sync.dma_start(
                                    out=ct[:, 0:gw, :],
                                    in_=gates_in["c" + tname][gcol:gcol + cw, :].rearrange(brearr, p=P))
                                nc.sync.dma_start(
                                    out=it[:, 0:gw, :],
                                    in_=gates_in["i" + tname][gcol:gcol + cw, :].rearrange(brearr, p=P))
                                nc.sync.dma_start(
                                    out=ftl[:, 0:gw, :],
                                    in_=gates_in["f" + tname][gcol:gcol + cw, :].rearrange(brearr, p=P))
                                nc.vector.tensor_mul(out=ftl[:, 0:gw, :], in0=ftl[:, 0:gw, :],
                                                     in1=ct[:, 0:gw, :])
                                nc.vector.tensor_mul(out=tns[t][:, 0:gw, :], in0=tns[t][:, 0:gw, :],
                                                     in1=it[:, 0:gw, :])
                                nc.vector.tensor_add(out=tns[t][:, 0:gw, :], in0=tns[t][:, 0:gw, :],
                                                     in1=ftl[:, 0:gw, :])
                                nc.sync.dma_start(
                                    out=outs[t][gcol:gcol + cw, :].rearrange(brearr, p=P),
                                    in_=tns[t][:, 0:gw, :])
                        # feature-major next-layer t (layer 0 only)
                        if not last:
                            for t, agl, tfx in ((0, (1, 2), tfa), (1, (0,), tfb)):
                                pF = (psFA if t == 0 else psFB).tile(
                                    [D, BL * P], f32, tag="pf", name=f"pF_{l}_{h}_{b}_{t}")
                                first = True
                                for e in agl:
                                    nc.tensor.matmul(out=pF[:, 0:cw],
                                                     lhsT=wn_t[:, l * 3 + e, :],
                                                     rhs=aggf[e][0:D, c0:c0 + cw],
                                                     start=first, stop=False)
                                    first = False
                                nc.tensor.matmul(out=pF[:, 0:cw],
                                                 lhsT=wr_t[:, l * 2 + t, :],
                                                 rhs=tfx[:, 0:cw],
                                                 start=False, stop=True)
                                tfo = dpool.tile([D, BL * P], bf16, tag=f"tfo{t}",
                                                 name=f"tfo_{l}_{h}_{b}_{t}")
                                nc.scalar.activation(
                                    tfo[:, 0:cw], pF[:, 0:cw],
                                    mybir.ActivationFunctionType.Identity,
                                    bias=blf_t[:, t:t + 1])
                                nc.sync.dma_start(out=tf1[t][:, gcol:gcol + cw],
                                                  in_=tfo[:, 0:cw])

                if not last:
                    for t in range(2):
                        if PAD > SH:
                            nc.sync.dma_start(out=stg[t][SH:PAD, :],
                                              in_=zero_small[0:PAD - SH, :])
                        if skip_cc:
                            nc.sync.dma_start(out=tab1[t][0:PAD, :], in_=stg[t][:])
                        else:
                            nc.gpsimd.collective_compute(
                                "AllGather", mybir.AluOpType.bypass,
                                replica_groups=[list(range(NCO))],
                                ins=[stg[t][:]], outs=[tab1[t][:]])

    # Align SWDGE queue_num with Tile's DMASW semaphore lane assignment:
    # each DMASW sem must only ever be updated from one SWDGE queue, and
    # Tile assigns lanes round-robin over the scheduled order. queue = lane%4.
    import re as _re
    qcount = {}
    for _ins in list(nc.inst_map.values()):
        if isinstance(_ins, (mybir.InstDMAGatherAnt, mybir.InstDMAScatterAddAnt)):
            _si = _ins.sync_info
            for _u in (_si.on_update or []):
                _m = _re.match(r"DMASW(\d+)", getattr(_u, "ant_name", "") or "")
                if _m:
                    _ins.queue_num = int(_m.group(1)) % 4
                    qcount[_ins.queue_num] = qcount.get(_ins.queue_num, 0) + 1
                    break
    nc._swdge_queue_dist = qcount

    nc.compile()
    return nc


# ---------------------------------------------------------------- host wrapper

def make_in_maps(cfg, inputs, percore):
    import ml_dtypes
    bf16 = ml_dtypes.bfloat16
    cfg = cfg_derived(cfg)
    NCO, SH, PAD, TROWS = cfg["n_cores"], cfg["shard"], cfg["pad"], cfg["trows"]

    def pad_rows(a):
        out = np.zeros((PAD, D), np.float32)
        out[:SH] = a
        return out

    # full x in table layout
    tabx = {}
    for t, xn in (("A", "x_A"), ("B", "x_B")):
        tb = np.zeros((TROWS, D), np.float32)
        x = np.asarray(inputs[xn], np.float32)
        for c in range(NCO):
            tb[PAD * c:PAD * c + SH] = x[SH * c:SH * (c + 1)]
        tabx[t] = tb

    Wl = np.asarray(inputs["Wl"], np.float32)
    Wr = np.asarray(inputs["Wr"], np.float32)
    bl = np.asarray(inputs["bl"], np.float32)
    WxA = np.asarray(inputs["Wx_A"], np.float32)
    WxB = np.asarray(inputs["Wx_B"], np.float32)
    biasA = np.asarray(inputs["bias_A"], np.float32)
    biasB = np.asarray(inputs["bias_B"], np.float32)
    Wx = [WxA, WxB]

    # wnode[l, e] = lhs-folded (Wl[l,e] @ Wx_src)^T for l==0 else Wl[l,e]^T
    wnode = np.zeros((L, 3, D, D), np.float32)
    for l in range(L):
        for e, (_, sT, dT) in enumerate(ETS):
            w = Wl[l, e] @ Wx[sT] if l == 0 else Wl[l, e]
            wnode[l, e] = w.T
    # wrn[l, t]: dst-type folded Wr sums
    wrn = np.zeros((L, 2, D, D), np.float32)
    for l in range(L):
        wrA = Wr[l, 1] + Wr[l, 2]
        wrB = Wr[l, 0]
        if l == 0:
            wrA = wrA @ WxA
            wrB = wrB @ WxB
        wrn[l, 0] = wrA.T
        wrn[l, 1] = wrB.T

    # biases: layer bl sums per dst type (+ final per-type bias at last layer)
    bl_t = np.zeros((L, 2, D), np.float32)
    for l in range(L):
        bl_t[l, 0] = bl[l, 1] + bl[l, 2]
        bl_t[l, 1] = bl[l, 0]
    bl_t[L - 1, 0] += biasA
    bl_t[L - 1, 1] += biasB
    biasrep = np.broadcast_to(bl_t[:, :, None, :], (L, 2, P, D)).copy()
    blf = np.ascontiguousarray(bl_t[0].T)  # [D, 2] (layer-0 feat-major bias)

    mz = np.zeros((PAD, P), bf16)

    in_maps = []
    for c in range(NCO):
        sl = slice(SH * c, SH * (c + 1))
        deg = percore[c]["deg"]  # [3, PAD] int32
        recipf = np.zeros((3, D, PAD), np.float32)
        recipf[:, :, :] = (1.0 / np.maximum(deg, 1.0))[:, None, :]
        xf = {}
        for t, xn in (("A", "x_A"), ("B", "x_B")):
            a = np.zeros((D, PAD), np.float32)
            a[:, :SH] = np.asarray(inputs[xn], np.float32)[sl].T
            xf[t] = a.astype(bf16)
        m = {
            "tabxA": tabx["A"], "tabxB": tabx["B"],
            "xfA": xf["A"], "xfB": xf["B"],
            "wnode": wnode.astype(bf16), "wrn": wrn.astype(bf16),
            "biasrep": biasrep, "blf": blf,
            "recipf": recipf.astype(bf16),
            "gidx": percore[c]["gidx"],
            "sidx": percore[c]["sidx"],
        }
        for l in range(L):
            for e in range(3):
                m[f"m{e}_{l}"] = mz
        for t in "AB":
            for nmm in "cif":
                m[f"{nmm}{t}"] = pad_rows(np.asarray(inputs[f"{nmm}_{t}"])[sl])
        in_maps.append(m)
    return in_maps


_BUILT = {}


def kernel(**inputs):
    from concourse.bass_utils import run_bass_kernel_spmd

    cfg0 = full_cfg()
    edges = {"AB": np.asarray(inputs["edge_AB"]),
             "BA": np.asarray(inputs["edge_BA"]),
             "AA": np.asarray(inputs["edge_AA"])}
    cfg, schedule, percore, cbtot = host_prep(cfg0, edges)

    key = (cbtot, tuple(schedule))
    if key not in _BUILT:
        _BUILT.clear()
        _BUILT[key] = build(cfg0, schedule, cbtot)
    nc = _BUILT[key]

    in_maps = make_in_maps(cfg0, inputs, percore)
    r = run_bass_kernel_spmd(nc, in_maps, core_ids=list(range(cfg["n_cores"])))

    SH = cfg["shard"]
    out_A = np.concatenate([r.results[c]["outA"][:SH] for c in range(cfg["n_cores"])], axis=0)
    out_B = np.concatenate([r.results[c]["outB"][:SH] for c in range(cfg["n_cores"])], axis=0)
    return (out_A, out_B)
